# revision 27
# baseline (speedup 1.0000x reference)
"""Batched Kalman-gain kernel for Trainium2 (Bass/Tile), 8-core data parallel.

Per batch b (262144 of them):
    Sigma = F Sp F^T + Q            [8,8]
    S     = H Sigma H^T + R         [4,4]
    KG    = Sigma H^T S^-1          [8,4]

Factored (A = H F, U = A Sp):
    P12 = F U^T + (H Q)^T  (= Sigma H^T)  [8,4]
    S   = H P12 + R                       [4,4]
    X   = S^-1  (SPD, 2x2-block Schur complement, fp32)
    KG  = P12 X                           [8,4]

Layout: "batch-innermost planes". The HOST pre-transposes every input to
[chunk, P, d0, d1, g] (P=128 SBUF partitions, g=64 consecutive batches
innermost) and casts to fp16. On device every per-batch product is a wide
elementwise DVE tensor_tensor in fp16 — the innermost g axis is stride-1 on
all operands, which enables the DVE 16-bit fast mode (~0.52 ns/free-elem
marginal + ~150 ns/op fixed, hence g=64 over g=32). Contraction sums for
A/U/P12/KG ride the TensorEngine (fp16 identity stationary, fp32 PSUM
accumulate, ~375 ns/512-col pass; ScalarE evacuates PSUM -> fp16). The HQ
products feed P12's PSUM directly through transposed access patterns. S
also sums on the PE (fp32 PSUM); DVE adds R while reading PSUM directly,
so S reaches the inverse in fp32. The 4x4 SPD inverse runs in fp32 on DVE,
batched over IBATCH chunks to amortize per-op overhead. GPSIMD is left idle
on purpose: concurrent GPSIMD traffic inflates DVE op durations ~40%
(SBUF port contention, measured).

Numerics (validated against a float64 reference in numpy): fp16 inputs +
fp16 products + fp32 PSUM sums + fp16 intermediates + fp32 S/inverse +
fp16 X/KG gives rel err ~3e-3 (tolerance 2e-2).
"""

import numpy as np

P = 128
G = 64
B = 262144
NCORES = 8
B_CORE = B // NCORES           # 32768
CHUNK = P * G                  # 8192
NCHUNK = B_CORE // CHUNK       # 4
IBATCH = 2                     # chunks per inverse batch

_NC_CACHE = {}


def _build_nc():
    import concourse.bacc as bacc
    import concourse.mybir as mybir
    import concourse.tile as tile
    from concourse.masks import make_identity

    fp32 = mybir.dt.float32
    fp16 = mybir.dt.float16
    MULT = mybir.AluOpType.mult
    ADD = mybir.AluOpType.add
    SUB = mybir.AluOpType.subtract
    COPY = mybir.ActivationFunctionType.Copy

    nc = bacc.Bacc("TRN2", target_bir_lowering=False, debug=False)

    F_d = nc.dram_tensor("F", [NCHUNK, P, 8, 8, G], fp16, kind="ExternalInput").ap()
    Sp_d = nc.dram_tensor(
        "Sigma_previous", [NCHUNK, P, 8, 8, G], fp16, kind="ExternalInput"
    ).ap()
    Q_d = nc.dram_tensor("Q", [NCHUNK, P, 8, 8, G], fp16, kind="ExternalInput").ap()
    H_d = nc.dram_tensor("H", [NCHUNK, P, 4, 8, G], fp16, kind="ExternalInput").ap()
    R_d = nc.dram_tensor("R", [NCHUNK, P, 4, 4, G], fp16, kind="ExternalInput").ap()
    KG_d = nc.dram_tensor("KG", [NCHUNK, P, 8, 4, G], fp16, kind="ExternalOutput").ap()

    NB = IBATCH
    W = 32 * G          # free elems per 32-entry matrix family (2048)
    NBANK = W // 512    # PSUM banks per family (4)

    with tile.TileContext(nc) as tc:
        with (
            tc.tile_pool(name="consts", bufs=1) as consts,
            tc.tile_pool(name="inF", bufs=4) as poolF,
            tc.tile_pool(name="inH", bufs=5) as poolH,
            tc.tile_pool(name="inSp", bufs=3) as poolSp,
            tc.tile_pool(name="inQ", bufs=2) as poolQ,
            tc.tile_pool(name="inR", bufs=2) as poolR,
            tc.tile_pool(name="pp", bufs=8) as ppool,
            tc.tile_pool(name="tprod", bufs=1) as tprod,
            tc.tile_pool(name="interm", bufs=2) as interm,
            tc.tile_pool(name="p12p", bufs=4) as p12p,
            tc.tile_pool(name="sx", bufs=2) as sxp,
            tc.tile_pool(name="inv", bufs=1) as invp,
            tc.tile_pool(name="out", bufs=2) as outp,
            tc.tile_pool(name="psum", bufs=2, space="PSUM") as psump,
        ):
            identf = consts.tile([P, P], fp32, tag="identf")
            make_identity(nc, identf[:])
            identh_t = consts.tile([P, P], fp16, tag="identh")
            nc.vector.tensor_copy(identh_t[:], identf[:])
            identh = identh_t[:]

            V = nc.vector
            ACT = nc.scalar

            def bc(ap, axis, shape):
                return ap.unsqueeze(axis).broadcast_to(shape)

            st = [dict() for _ in range(NCHUNK)]
            inv_st = [dict() for _ in range(NCHUNK // NB)]

            sh48 = [P, 4, 8, G]
            sh84 = [P, 8, 4, G]
            sh44 = [P, 4, 4, G]

            def emit_load(c):
                s = st[c]
                s["F"] = poolF.tile([P, 8, 8, G], fp16, tag="F", name="Ft")
                s["H"] = poolH.tile([P, 4, 8, G], fp16, tag="H", name="Ht")
                nc.sync.dma_start(out=s["H"][:], in_=H_d[c])
                if c == 0:
                    # split: A(0)'s first products only need F rows 0-3
                    nc.sync.dma_start(out=s["F"][:, 0:4], in_=F_d[c][:, 0:4])
                    nc.sync.dma_start(out=s["F"][:, 4:8], in_=F_d[c][:, 4:8])
                else:
                    nc.sync.dma_start(out=s["F"][:], in_=F_d[c])

            def emit_load_Sp(c):
                s = st[c]
                s["Sp"] = poolSp.tile([P, 8, 8, G], fp16, tag="Sp", name="Spt")
                nc.sync.dma_start(out=s["Sp"][:], in_=Sp_d[c])

            def emit_load_QR(c):
                s = st[c]
                s["Q"] = poolQ.tile([P, 8, 8, G], fp16, tag="Q", name="Qt")
                s["R"] = poolR.tile([P, 4, 4, G], fp16, tag="R", name="Rt")
                nc.sync.dma_start(out=s["Q"][:], in_=Q_d[c])
                nc.sync.dma_start(out=s["R"][:], in_=R_d[c])

            def pslot(shape):
                t = ppool.tile([P, W], fp16, tag="pp", name="pp")
                n = shape[1] * shape[2] * G
                return t[:][:, :n].rearrange(
                    "p (a b g) -> p a b g", a=shape[1], b=shape[2], g=G
                )

            def banks(ap4):
                # [P, d0, d1, G] -> NBANK 512-elem bank APs, split along d0
                d0 = ap4.shape[1]
                step = d0 // NBANK
                return [ap4[:, i * step : (i + 1) * step] for i in range(NBANK)]

            def pe_contract(slots, out_tag, pool):
                """slots: list of per-term [P, W]-wide fp16 moving APs.
                One W-wide matmul per term (spans W//512 PSUM banks), one
                W-wide ACT evacuation. Returns fp16 SBUF tile [P, W] flat."""
                nterm = len(slots)
                ps = psump.tile([P, W], fp32, tag="ps", name=f"ps_{out_tag}")
                for t, mv in enumerate(slots):
                    for b in range(NBANK):
                        nc.tensor.matmul(
                            ps[:, b * 512 : (b + 1) * 512],
                            identh,
                            mv[b],
                            start=(t == 0),
                            stop=(t == nterm - 1),
                        )
                out = pool.tile([P, W], fp16, tag=out_tag, name=out_tag)
                ACT.activation(out[:], ps[:, :], COPY)
                return out

            def emit_A(c):
                s = st[c]
                Ft, Ht = s["F"], s["H"]
                slots = []
                for j in range(8):
                    pv = pslot(sh48)
                    V.tensor_tensor(
                        pv,
                        bc(Ht[:, :, j, :], 2, sh48),
                        bc(Ft[:, j, :, :], 1, sh48),
                        op=MULT,
                    )
                    slots.append(banks(pv))
                s["A"] = pe_contract(slots, "A", interm)  # A[m,k]

            def emit_U(c):
                s = st[c]
                Spt = s["Sp"]
                Av = s["A"][:].rearrange("p (m k g) -> p m k g", m=4, k=8)
                slots = []
                for k in range(8):
                    pv = pslot(sh48)
                    V.tensor_tensor(
                        pv,
                        bc(Av[:, :, k, :], 2, sh48),
                        bc(Spt[:, k, :, :], 1, sh48),
                        op=MULT,
                    )
                    slots.append(banks(pv))
                s["U"] = pe_contract(slots, "U", interm)  # U[m,j]

            def emit_P12(c):
                s = st[c]
                Ft, Ht, Qt = s["F"], s["H"], s["Q"]
                Uv = s["U"][:].rearrange("p (m j g) -> p m j g", m=4, j=8)
                slots = []
                for j in range(8):
                    pv = pslot(sh84)
                    V.tensor_tensor(
                        pv,
                        bc(Ft[:, :, j, :], 2, sh84),
                        bc(Uv[:, :, j, :], 1, sh84),
                        op=MULT,
                    )
                    slots.append(banks(pv))
                # HQ products [m,k], fed transposed ([k,m]) into the same PSUM
                for j in range(8):
                    pv = pslot(sh48)
                    V.tensor_tensor(
                        pv,
                        bc(Ht[:, :, j, :], 2, sh48),
                        bc(Qt[:, j, :, :], 1, sh48),
                        op=MULT,
                    )
                    slots.append(banks(pv.transpose([0, 2, 1, 3])))
                s["P12"] = pe_contract(slots, "P12", p12p)

            def emit_S(c):
                s = st[c]
                Ht, Rt = s["H"], s["R"]
                P12v = s["P12"][:].rearrange("p (i m g) -> p i m g", i=8, m=4)
                WS = 16 * G
                NB2 = WS // 512
                ps = psump.tile([P, W], fp32, tag="ps", name="ps_S")
                for i in range(8):
                    pv = pslot(sh44)
                    V.tensor_tensor(
                        pv,
                        bc(Ht[:, :, i, :], 2, sh44),
                        bc(P12v[:, i, :, :], 1, sh44),
                        op=MULT,
                    )
                    for b in range(NB2):
                        nc.tensor.matmul(
                            ps[:, b * 512 : (b + 1) * 512],
                            identh,
                            pv[:, b * 2 : (b + 1) * 2],
                            start=(i == 0),
                            stop=False,
                        )
                # 9th term: R rides the PE too; ACT evacuates S4 (fp32) direct
                for b in range(NB2):
                    nc.tensor.matmul(
                        ps[:, b * 512 : (b + 1) * 512],
                        identh,
                        Rt[:, b * 2 : (b + 1) * 2],
                        start=False,
                        stop=True,
                    )
                k, ci = c // NB, c % NB
                if ci == 0:
                    inv_st[k]["S4"] = sxp.tile(
                        [P, 4, 4, NB, G], fp32, tag="S4", name="S4"
                    )
                S4 = inv_st[k]["S4"]
                ACT.activation(
                    S4[:, :, :, ci, :],
                    ps[:, :WS].rearrange("p (m n g) -> p m n g", m=4, n=4),
                    COPY,
                )

            def emit_INV(k):
                """X4 = S4^-1 via Schur complement of leading 2x2 block.
                S treated as symmetric (s10 := s01). All internals fp32,
                X4 written fp16. Scratch tags reused across steps (bufs=1)."""
                s = inv_st[k]
                S4 = s["S4"]
                X4 = sxp.tile([P, 4, 4, NB, G], fp16, tag="X4", name="X4")
                s["X4"] = X4
                sh1 = [P, NB, G]
                sh2 = [P, 2, 2, NB, G]

                def t1(tag):
                    return invp.tile(sh1, fp32, tag=tag, name=tag)

                def t2(tag):
                    return invp.tile(sh2, fp32, tag=tag, name=tag)

                sa, sb, sc_ = S4[:, 0, 0], S4[:, 0, 1], S4[:, 1, 1]
                ta, tb, td, tr, tnr = (
                    t1("ta"), t1("tb"), t1("td"), t1("tr"), t1("tnr"),
                )
                V.tensor_tensor(ta[:], sa, sc_, op=MULT)
                V.tensor_tensor(tb[:], sb, sb, op=MULT)
                V.tensor_tensor(td[:], ta[:], tb[:], op=SUB)
                V.reciprocal_approx_fast(tr[:], td[:])
                V.tensor_scalar_mul(tnr[:], tr[:], -1.0)
                Pi = t2("Pi")
                V.tensor_tensor(Pi[:, 0, 0], sc_, tr[:], op=MULT)
                V.tensor_tensor(Pi[:, 1, 1], sa, tr[:], op=MULT)
                V.tensor_tensor(Pi[:, 0, 1], sb, tnr[:], op=MULT)
                V.tensor_copy(Pi[:, 1, 0], Pi[:, 0, 1])
                Bq = S4[:, 2:4, 0:2]
                Wt, x2a = t2("W"), t2("x2a")
                V.tensor_tensor(
                    x2a[:], bc(Bq[:, :, 0], 2, sh2), bc(Pi[:, 0, :], 1, sh2), op=MULT
                )
                V.tensor_tensor(
                    Wt[:], bc(Bq[:, :, 1], 2, sh2), bc(Pi[:, 1, :], 1, sh2), op=MULT
                )
                V.tensor_tensor(Wt[:], Wt[:], x2a[:], op=ADD)
                x2b, Sc = t2("x2b"), t2("Sc")
                V.tensor_tensor(
                    x2a[:], bc(Wt[:, :, 0], 2, sh2), bc(Bq[:, :, 0], 1, sh2), op=MULT
                )
                V.tensor_tensor(
                    x2b[:], bc(Wt[:, :, 1], 2, sh2), bc(Bq[:, :, 1], 1, sh2), op=MULT
                )
                V.tensor_tensor(x2a[:], x2a[:], x2b[:], op=ADD)
                V.tensor_tensor(Sc[:], S4[:, 2:4, 2:4], x2a[:], op=SUB)
                V.tensor_tensor(ta[:], Sc[:, 0, 0], Sc[:, 1, 1], op=MULT)
                V.tensor_tensor(tb[:], Sc[:, 0, 1], Sc[:, 1, 0], op=MULT)
                V.tensor_tensor(td[:], ta[:], tb[:], op=SUB)
                V.reciprocal_approx_fast(tr[:], td[:])
                V.tensor_scalar_mul(tnr[:], tr[:], -1.0)
                Si = t2("Si")
                V.tensor_tensor(Si[:, 0, 0], Sc[:, 1, 1], tr[:], op=MULT)
                V.tensor_tensor(Si[:, 1, 1], Sc[:, 0, 0], tr[:], op=MULT)
                V.tensor_tensor(Si[:, 0, 1], Sc[:, 0, 1], tnr[:], op=MULT)
                V.tensor_copy(Si[:, 1, 0], Si[:, 0, 1])
                V.tensor_copy(X4[:, 2:4, 2:4], Si[:])
                x2c = t2("Sc")  # Sc is dead past this point; reuse its buffer
                V.tensor_tensor(
                    x2a[:], bc(Si[:, :, 0], 2, sh2), bc(Wt[:, 0, :], 1, sh2), op=MULT
                )
                V.tensor_tensor(
                    x2b[:], bc(Si[:, :, 1], 2, sh2), bc(Wt[:, 1, :], 1, sh2), op=MULT
                )
                V.tensor_tensor(x2a[:], x2a[:], x2b[:], op=ADD)
                V.tensor_scalar_mul(X4[:, 2:4, 0:2], x2a[:], -1.0)
                V.tensor_copy(
                    X4[:, 0:2, 2:4], X4[:, 2:4, 0:2].transpose([0, 2, 1, 3, 4])
                )
                V.tensor_tensor(
                    x2b[:], bc(Wt[:, 0, :], 2, sh2), bc(x2a[:, 0, :], 1, sh2), op=MULT
                )
                V.tensor_tensor(
                    x2c[:], bc(Wt[:, 1, :], 2, sh2), bc(x2a[:, 1, :], 1, sh2), op=MULT
                )
                V.tensor_tensor(x2b[:], x2b[:], x2c[:], op=ADD)
                V.tensor_tensor(X4[:, 0:2, 0:2], Pi[:], x2b[:], op=ADD)

            def emit_KG(c):
                s = st[c]
                k, ci = c // NB, c % NB
                X4 = inv_st[k]["X4"]
                P12v = s["P12"][:].rearrange("p (i m g) -> p i m g", i=8, m=4)
                slots = []
                for m in range(4):
                    pv = pslot(sh84)
                    V.tensor_tensor(
                        pv,
                        bc(P12v[:, :, m, :], 2, sh84),
                        bc(X4[:, m, :, ci, :], 1, sh84),
                        op=MULT,
                    )
                    slots.append(banks(pv))
                ps = psump.tile([P, W], fp32, tag="ps", name="ps_KG")
                for t, mv in enumerate(slots):
                    for b in range(NBANK):
                        nc.tensor.matmul(
                            ps[:, b * 512 : (b + 1) * 512], identh, mv[b],
                            start=(t == 0), stop=(t == 3),
                        )
                KGh = outp.tile([P, 8, 4, G], fp16, tag="KGh", name="KGh")
                ACT.activation(KGh[:].rearrange("p i n g -> p (i n g)"), ps[:, :], COPY)
                nc.sync.dma_start(out=KG_d[c], in_=KGh[:])

            for t in range(NCHUNK + 6):
                if t < NCHUNK:
                    emit_load(t)
                if t < NCHUNK:
                    emit_load_Sp(t)
                if 0 <= t - 2 < NCHUNK:
                    emit_load_QR(t - 2)
                if 0 <= t - 1 < NCHUNK:
                    emit_A(t - 1)
                if 0 <= t - 2 < NCHUNK:
                    emit_U(t - 2)
                if 0 <= t - 3 < NCHUNK:
                    emit_P12(t - 3)
                if 0 <= t - 4 < NCHUNK:
                    emit_S(t - 4)
                if 0 <= t - 4 < NCHUNK and (t - 4) % NB == NB - 1:
                    emit_INV((t - 4) // NB)
                if 0 <= t - 5 < NCHUNK:
                    emit_KG(t - 5)

    nc.compile()
    return nc


def _get_nc():
    if "nc" not in _NC_CACHE:
        _NC_CACHE["nc"] = _build_nc()
    return _NC_CACHE["nc"]


def prepare_in_map(F, H, Sigma_previous, Q, R, core):
    """Host-side shard + layout transform + fp16 cast for one core."""
    sl = slice(core * B_CORE, (core + 1) * B_CORE)

    def bi(x, d0, d1):
        # [B_CORE, d0, d1] -> [NCHUNK, P, d0, d1, G]
        v = x[sl].reshape(NCHUNK, P, G, d0, d1).transpose(0, 1, 3, 4, 2)
        return np.ascontiguousarray(v, dtype=np.float16)

    return {
        "F": bi(F, 8, 8),
        "Sigma_previous": bi(Sigma_previous, 8, 8),
        "Q": bi(Q, 8, 8),
        "H": bi(H, 4, 8),
        "R": bi(R, 4, 4),
    }


def postprocess(results):
    """[per-core dicts with KG [NCHUNK, P, 8, 4, G] fp16] -> [B, 8, 4] fp32."""
    outs = []
    for r in results:
        kg = r["KG"].astype(np.float32)          # [NCHUNK, P, 8, 4, G]
        kg = kg.transpose(0, 1, 4, 2, 3).reshape(B_CORE, 8, 4)
        outs.append(kg)
    return np.concatenate(outs, axis=0)


def kernel(F, H, Sigma_previous, Q, R):
    from concourse.bass_utils import run_bass_kernel_spmd

    nc = _get_nc()
    in_maps = [
        prepare_in_map(F, H, Sigma_previous, Q, R, ci) for ci in range(NCORES)
    ]
    res = run_bass_kernel_spmd(nc, in_maps, core_ids=list(range(NCORES)))
    return postprocess(res.results)


# revision 28
# speedup vs baseline: 1.0361x; 1.0361x over previous
"""Batched Kalman-gain kernel for Trainium2 (Bass/Tile), 8-core data parallel.

Per batch b (262144 of them):
    Sigma = F Sp F^T + Q            [8,8]
    S     = H Sigma H^T + R         [4,4]
    KG    = Sigma H^T S^-1          [8,4]

Factored (A = H F, U = A Sp):
    P12 = F U^T + (H Q)^T  (= Sigma H^T)  [8,4]
    S   = H P12 + R                       [4,4]
    X   = S^-1  (SPD, 2x2-block Schur complement, fp32)
    KG  = P12 X                           [8,4]

Layout: "batch-innermost planes". The HOST pre-transposes every input to
[chunk, P, d0, d1, g] (P=128 SBUF partitions, g=64 consecutive batches
innermost) and casts to fp16. On device every per-batch product is a wide
elementwise DVE tensor_tensor in fp16 — the innermost g axis is stride-1 on
all operands, which enables the DVE 16-bit fast mode (~0.52 ns/free-elem
marginal + ~150 ns/op fixed, hence g=64 over g=32). Contraction sums for
A/U/P12/KG ride the TensorEngine (fp16 identity stationary, fp32 PSUM
accumulate, ~375 ns/512-col pass; ScalarE evacuates PSUM -> fp16). The HQ
products feed P12's PSUM directly through transposed access patterns. S
also sums on the PE (fp32 PSUM); DVE adds R while reading PSUM directly,
so S reaches the inverse in fp32. The 4x4 SPD inverse runs in fp32 on DVE,
batched over IBATCH chunks to amortize per-op overhead. GPSIMD is left idle
on purpose: concurrent GPSIMD traffic inflates DVE op durations ~40%
(SBUF port contention, measured).

Numerics (validated against a float64 reference in numpy): fp16 inputs +
fp16 products + fp32 PSUM sums + fp16 intermediates + fp32 S/inverse +
fp16 X/KG gives rel err ~3e-3 (tolerance 2e-2).
"""

import numpy as np

P = 128
G = 64
B = 262144
NCORES = 8
B_CORE = B // NCORES           # 32768
CHUNK = P * G                  # 8192
NCHUNK = B_CORE // CHUNK       # 4
IBATCH = 2                     # chunks per inverse batch

_NC_CACHE = {}


def _build_nc():
    import concourse.bacc as bacc
    import concourse.mybir as mybir
    import concourse.tile as tile
    from concourse.masks import make_identity

    fp32 = mybir.dt.float32
    fp16 = mybir.dt.float16
    MULT = mybir.AluOpType.mult
    ADD = mybir.AluOpType.add
    SUB = mybir.AluOpType.subtract
    COPY = mybir.ActivationFunctionType.Copy

    nc = bacc.Bacc("TRN2", target_bir_lowering=False, debug=False)

    F_d = nc.dram_tensor("F", [NCHUNK, P, 8, 8, G], fp16, kind="ExternalInput").ap()
    Sp_d = nc.dram_tensor(
        "Sigma_previous", [NCHUNK, P, 8, 8, G], fp16, kind="ExternalInput"
    ).ap()
    Q_d = nc.dram_tensor("Q", [NCHUNK, P, 8, 8, G], fp16, kind="ExternalInput").ap()
    H_d = nc.dram_tensor("H", [NCHUNK, P, 4, 8, G], fp16, kind="ExternalInput").ap()
    R_d = nc.dram_tensor("R", [NCHUNK, P, 4, 4, G], fp16, kind="ExternalInput").ap()
    KG_d = nc.dram_tensor("KG", [NCHUNK, P, 8, 4, G], fp16, kind="ExternalOutput").ap()

    NB = IBATCH
    W = 32 * G          # free elems per 32-entry matrix family (2048)
    NBANK = W // 512    # PSUM banks per family (4)

    with tile.TileContext(nc) as tc:
        with (
            tc.tile_pool(name="consts", bufs=1) as consts,
            tc.tile_pool(name="inF", bufs=4) as poolF,
            tc.tile_pool(name="inH", bufs=5) as poolH,
            tc.tile_pool(name="inSp", bufs=3) as poolSp,
            tc.tile_pool(name="inQ", bufs=2) as poolQ,
            tc.tile_pool(name="inR", bufs=2) as poolR,
            tc.tile_pool(name="pp", bufs=8) as ppool,
            tc.tile_pool(name="tprod", bufs=1) as tprod,
            tc.tile_pool(name="interm", bufs=2) as interm,
            tc.tile_pool(name="p12p", bufs=4) as p12p,
            tc.tile_pool(name="sx", bufs=2) as sxp,
            tc.tile_pool(name="inv", bufs=1) as invp,
            tc.tile_pool(name="out", bufs=2) as outp,
            tc.tile_pool(name="psum", bufs=2, space="PSUM") as psump,
        ):
            identf = consts.tile([P, P], fp32, tag="identf")
            make_identity(nc, identf[:])
            identh_t = consts.tile([P, P], fp16, tag="identh")
            nc.vector.tensor_copy(identh_t[:], identf[:])
            identh = identh_t[:]

            V = nc.vector
            ACT = nc.scalar

            def bc(ap, axis, shape):
                return ap.unsqueeze(axis).broadcast_to(shape)

            st = [dict() for _ in range(NCHUNK)]
            inv_st = [dict() for _ in range(NCHUNK // NB)]

            sh48 = [P, 4, 8, G]
            sh84 = [P, 8, 4, G]
            sh44 = [P, 4, 4, G]

            def emit_load(c):
                s = st[c]
                s["F"] = poolF.tile([P, 8, 8, G], fp16, tag="F", name="Ft")
                s["H"] = poolH.tile([P, 4, 8, G], fp16, tag="H", name="Ht")
                nc.sync.dma_start(out=s["H"][:], in_=H_d[c])
                if c == 0:
                    # split: A(0)'s first products only need F rows 0-3
                    nc.sync.dma_start(out=s["F"][:, 0:4], in_=F_d[c][:, 0:4])
                    nc.sync.dma_start(out=s["F"][:, 4:8], in_=F_d[c][:, 4:8])
                else:
                    nc.sync.dma_start(out=s["F"][:], in_=F_d[c])

            def emit_load_Sp(c):
                s = st[c]
                s["Sp"] = poolSp.tile([P, 8, 8, G], fp16, tag="Sp", name="Spt")
                nc.sync.dma_start(out=s["Sp"][:], in_=Sp_d[c])

            def emit_load_QR(c):
                s = st[c]
                s["Q"] = poolQ.tile([P, 8, 8, G], fp16, tag="Q", name="Qt")
                s["R"] = poolR.tile([P, 4, 4, G], fp16, tag="R", name="Rt")
                nc.sync.dma_start(out=s["Q"][:], in_=Q_d[c])
                nc.sync.dma_start(out=s["R"][:], in_=R_d[c])

            def pslot(shape):
                t = ppool.tile([P, W], fp16, tag="pp", name="pp")
                n = shape[1] * shape[2] * G
                return t[:][:, :n].rearrange(
                    "p (a b g) -> p a b g", a=shape[1], b=shape[2], g=G
                )

            def banks(ap4):
                # [P, d0, d1, G] -> NBANK 512-elem bank APs, split along d0
                d0 = ap4.shape[1]
                step = d0 // NBANK
                return [ap4[:, i * step : (i + 1) * step] for i in range(NBANK)]

            def pe_contract(slots, out_tag, pool):
                """slots: list of per-term [P, W]-wide fp16 moving APs.
                One W-wide matmul per term (spans W//512 PSUM banks), one
                W-wide ACT evacuation. Returns fp16 SBUF tile [P, W] flat."""
                nterm = len(slots)
                ps = psump.tile([P, W], fp32, tag="ps", name=f"ps_{out_tag}")
                for t, mv in enumerate(slots):
                    for b in range(NBANK):
                        nc.tensor.matmul(
                            ps[:, b * 512 : (b + 1) * 512],
                            identh,
                            mv[b],
                            start=(t == 0),
                            stop=(t == nterm - 1),
                        )
                out = pool.tile([P, W], fp16, tag=out_tag, name=out_tag)
                ACT.activation(out[:], ps[:, :], COPY)
                return out

            def emit_A(c):
                s = st[c]
                Ft, Ht = s["F"], s["H"]
                slots = []
                for j in range(8):
                    pv = pslot(sh48)
                    V.tensor_tensor(
                        pv,
                        bc(Ht[:, :, j, :], 2, sh48),
                        bc(Ft[:, j, :, :], 1, sh48),
                        op=MULT,
                    )
                    slots.append(banks(pv))
                s["A"] = pe_contract(slots, "A", interm)  # A[m,k]

            def emit_U(c):
                s = st[c]
                Spt = s["Sp"]
                Av = s["A"][:].rearrange("p (m k g) -> p m k g", m=4, k=8)
                slots = []
                for k in range(8):
                    pv = pslot(sh48)
                    V.tensor_tensor(
                        pv,
                        bc(Av[:, :, k, :], 2, sh48),
                        bc(Spt[:, k, :, :], 1, sh48),
                        op=MULT,
                    )
                    slots.append(banks(pv))
                s["U"] = pe_contract(slots, "U", interm)  # U[m,j]

            def emit_P12(c):
                s = st[c]
                Ft, Ht, Qt = s["F"], s["H"], s["Q"]
                Uv = s["U"][:].rearrange("p (m j g) -> p m j g", m=4, j=8)
                slots = []
                for j in range(8):
                    pv = pslot(sh84)
                    V.tensor_tensor(
                        pv,
                        bc(Ft[:, :, j, :], 2, sh84),
                        bc(Uv[:, :, j, :], 1, sh84),
                        op=MULT,
                    )
                    slots.append(banks(pv))
                # HQ products [m,k], fed transposed ([k,m]) into the same PSUM
                for j in range(8):
                    pv = pslot(sh48)
                    V.tensor_tensor(
                        pv,
                        bc(Ht[:, :, j, :], 2, sh48),
                        bc(Qt[:, j, :, :], 1, sh48),
                        op=MULT,
                    )
                    slots.append(banks(pv.transpose([0, 2, 1, 3])))
                s["P12"] = pe_contract(slots, "P12", p12p)

            def emit_S(c):
                s = st[c]
                Ht, Rt = s["H"], s["R"]
                P12v = s["P12"][:].rearrange("p (i m g) -> p i m g", i=8, m=4)
                WS = 16 * G
                NB2 = WS // 512
                ps = psump.tile([P, W], fp32, tag="ps", name="ps_S")
                for i in range(8):
                    pv = pslot(sh44)
                    V.tensor_tensor(
                        pv,
                        bc(Ht[:, :, i, :], 2, sh44),
                        bc(P12v[:, i, :, :], 1, sh44),
                        op=MULT,
                    )
                    for b in range(NB2):
                        nc.tensor.matmul(
                            ps[:, b * 512 : (b + 1) * 512],
                            identh,
                            pv[:, b * 2 : (b + 1) * 2],
                            start=(i == 0),
                            stop=False,
                        )
                # 9th term: R rides the PE too; ACT evacuates S4 (fp32) direct
                for b in range(NB2):
                    nc.tensor.matmul(
                        ps[:, b * 512 : (b + 1) * 512],
                        identh,
                        Rt[:, b * 2 : (b + 1) * 2],
                        start=False,
                        stop=True,
                    )
                k, ci = c // NB, c % NB
                if ci == 0:
                    inv_st[k]["S4"] = sxp.tile(
                        [P, 4, 4, NB, G], fp32, tag="S4", name="S4"
                    )
                S4 = inv_st[k]["S4"]
                ACT.activation(
                    S4[:, :, :, ci, :],
                    ps[:, :WS].rearrange("p (m n g) -> p m n g", m=4, n=4),
                    COPY,
                )

            def emit_INV(k):
                """X4 = S4^-1 via Schur complement of leading 2x2 block.
                S treated as symmetric (s10 := s01). All internals fp32,
                X4 written fp16. Scratch tags reused across steps (bufs=1)."""
                s = inv_st[k]
                S4 = s["S4"]
                X4 = sxp.tile([P, 4, 4, NB, G], fp16, tag="X4", name="X4")
                s["X4"] = X4
                sh1 = [P, NB, G]
                sh2 = [P, 2, 2, NB, G]

                def t1(tag):
                    return invp.tile(sh1, fp32, tag=tag, name=tag)

                def t2(tag):
                    return invp.tile(sh2, fp32, tag=tag, name=tag)

                sa, sb, sc_ = S4[:, 0, 0], S4[:, 0, 1], S4[:, 1, 1]
                ta, tb, td, tr, tnr = (
                    t1("ta"), t1("tb"), t1("td"), t1("tr"), t1("tnr"),
                )
                V.tensor_tensor(ta[:], sa, sc_, op=MULT)
                V.tensor_tensor(tb[:], sb, sb, op=MULT)
                V.tensor_tensor(td[:], ta[:], tb[:], op=SUB)
                V.reciprocal_approx_fast(tr[:], td[:])
                V.tensor_scalar_mul(tnr[:], tr[:], -1.0)
                Pi = t2("Pi")
                V.tensor_tensor(Pi[:, 0, 0], sc_, tr[:], op=MULT)
                V.tensor_tensor(Pi[:, 1, 1], sa, tr[:], op=MULT)
                V.tensor_tensor(Pi[:, 0, 1], sb, tnr[:], op=MULT)
                V.tensor_copy(Pi[:, 1, 0], Pi[:, 0, 1])
                Bq = S4[:, 2:4, 0:2]
                Wt, x2a = t2("W"), t2("x2a")
                V.tensor_tensor(
                    x2a[:], bc(Bq[:, :, 0], 2, sh2), bc(Pi[:, 0, :], 1, sh2), op=MULT
                )
                V.tensor_tensor(
                    Wt[:], bc(Bq[:, :, 1], 2, sh2), bc(Pi[:, 1, :], 1, sh2), op=MULT
                )
                V.tensor_tensor(Wt[:], Wt[:], x2a[:], op=ADD)
                x2b, Sc = t2("x2b"), t2("Sc")
                V.tensor_tensor(
                    x2a[:], bc(Wt[:, :, 0], 2, sh2), bc(Bq[:, :, 0], 1, sh2), op=MULT
                )
                V.tensor_tensor(
                    x2b[:], bc(Wt[:, :, 1], 2, sh2), bc(Bq[:, :, 1], 1, sh2), op=MULT
                )
                V.tensor_tensor(x2a[:], x2a[:], x2b[:], op=ADD)
                V.tensor_tensor(Sc[:], S4[:, 2:4, 2:4], x2a[:], op=SUB)
                V.tensor_tensor(ta[:], Sc[:, 0, 0], Sc[:, 1, 1], op=MULT)
                V.tensor_tensor(tb[:], Sc[:, 0, 1], Sc[:, 1, 0], op=MULT)
                V.tensor_tensor(td[:], ta[:], tb[:], op=SUB)
                V.reciprocal_approx_fast(tr[:], td[:])
                V.tensor_scalar_mul(tnr[:], tr[:], -1.0)
                Si = t2("Si")
                V.tensor_tensor(Si[:, 0, 0], Sc[:, 1, 1], tr[:], op=MULT)
                V.tensor_tensor(Si[:, 1, 1], Sc[:, 0, 0], tr[:], op=MULT)
                V.tensor_tensor(Si[:, 0, 1], Sc[:, 0, 1], tnr[:], op=MULT)
                V.tensor_copy(Si[:, 1, 0], Si[:, 0, 1])
                V.tensor_copy(X4[:, 2:4, 2:4], Si[:])
                x2c = t2("Sc")  # Sc is dead past this point; reuse its buffer
                V.tensor_tensor(
                    x2a[:], bc(Si[:, :, 0], 2, sh2), bc(Wt[:, 0, :], 1, sh2), op=MULT
                )
                V.tensor_tensor(
                    x2b[:], bc(Si[:, :, 1], 2, sh2), bc(Wt[:, 1, :], 1, sh2), op=MULT
                )
                V.tensor_tensor(x2a[:], x2a[:], x2b[:], op=ADD)
                V.tensor_scalar_mul(X4[:, 2:4, 0:2], x2a[:], -1.0)
                V.tensor_copy(
                    X4[:, 0:2, 2:4], X4[:, 2:4, 0:2].transpose([0, 2, 1, 3, 4])
                )
                V.tensor_tensor(
                    x2b[:], bc(Wt[:, 0, :], 2, sh2), bc(x2a[:, 0, :], 1, sh2), op=MULT
                )
                V.tensor_tensor(
                    x2c[:], bc(Wt[:, 1, :], 2, sh2), bc(x2a[:, 1, :], 1, sh2), op=MULT
                )
                V.tensor_tensor(x2b[:], x2b[:], x2c[:], op=ADD)
                V.tensor_tensor(X4[:, 0:2, 0:2], Pi[:], x2b[:], op=ADD)

            def emit_KG(c):
                s = st[c]
                k, ci = c // NB, c % NB
                X4 = inv_st[k]["X4"]
                P12v = s["P12"][:].rearrange("p (i m g) -> p i m g", i=8, m=4)
                slots = []
                for m in range(4):
                    pv = pslot(sh84)
                    V.tensor_tensor(
                        pv,
                        bc(P12v[:, :, m, :], 2, sh84),
                        bc(X4[:, m, :, ci, :], 1, sh84),
                        op=MULT,
                    )
                    slots.append(banks(pv))
                ps = psump.tile([P, W], fp32, tag="ps", name="ps_KG")
                for t, mv in enumerate(slots):
                    for b in range(NBANK):
                        nc.tensor.matmul(
                            ps[:, b * 512 : (b + 1) * 512], identh, mv[b],
                            start=(t == 0), stop=(t == 3),
                        )
                KGh = outp.tile([P, 8, 4, G], fp16, tag="KGh", name="KGh")
                ACT.activation(KGh[:].rearrange("p i n g -> p (i n g)"), ps[:, :], COPY)
                nc.sync.dma_start(out=KG_d[c], in_=KGh[:])

            for t in range(NCHUNK + 7):
                if t < NCHUNK:
                    emit_load(t)
                if t < NCHUNK:
                    emit_load_Sp(t)
                if 0 <= t - 2 < NCHUNK:
                    emit_load_QR(t - 2)
                if 0 <= t - 1 < NCHUNK:
                    emit_A(t - 1)
                if 0 <= t - 2 < NCHUNK:
                    emit_U(t - 2)
                if 0 <= t - 3 < NCHUNK:
                    emit_P12(t - 3)
                if 0 <= t - 4 < NCHUNK:
                    emit_S(t - 4)
                if 0 <= t - 4 < NCHUNK and (t - 4) % NB == NB - 1:
                    emit_INV((t - 4) // NB)
                if 0 <= t - 6 < NCHUNK:
                    emit_KG(t - 6)

    nc.compile()
    return nc


def _get_nc():
    if "nc" not in _NC_CACHE:
        _NC_CACHE["nc"] = _build_nc()
    return _NC_CACHE["nc"]


def prepare_in_map(F, H, Sigma_previous, Q, R, core):
    """Host-side shard + layout transform + fp16 cast for one core."""
    sl = slice(core * B_CORE, (core + 1) * B_CORE)

    def bi(x, d0, d1):
        # [B_CORE, d0, d1] -> [NCHUNK, P, d0, d1, G]
        v = x[sl].reshape(NCHUNK, P, G, d0, d1).transpose(0, 1, 3, 4, 2)
        return np.ascontiguousarray(v, dtype=np.float16)

    return {
        "F": bi(F, 8, 8),
        "Sigma_previous": bi(Sigma_previous, 8, 8),
        "Q": bi(Q, 8, 8),
        "H": bi(H, 4, 8),
        "R": bi(R, 4, 4),
    }


def postprocess(results):
    """[per-core dicts with KG [NCHUNK, P, 8, 4, G] fp16] -> [B, 8, 4] fp32."""
    outs = []
    for r in results:
        kg = r["KG"].astype(np.float32)          # [NCHUNK, P, 8, 4, G]
        kg = kg.transpose(0, 1, 4, 2, 3).reshape(B_CORE, 8, 4)
        outs.append(kg)
    return np.concatenate(outs, axis=0)


def kernel(F, H, Sigma_previous, Q, R):
    from concourse.bass_utils import run_bass_kernel_spmd

    nc = _get_nc()
    in_maps = [
        prepare_in_map(F, H, Sigma_previous, Q, R, ci) for ci in range(NCORES)
    ]
    res = run_bass_kernel_spmd(nc, in_maps, core_ids=list(range(NCORES)))
    return postprocess(res.results)


# revision 29
# speedup vs baseline: 1.0377x; 1.0016x over previous
"""Batched Kalman-gain kernel for Trainium2 (Bass/Tile), 8-core data parallel.

Per batch b (262144 of them):
    Sigma = F Sp F^T + Q            [8,8]
    S     = H Sigma H^T + R         [4,4]
    KG    = Sigma H^T S^-1          [8,4]

Factored (A = H F, U = A Sp):
    P12 = F U^T + (H Q)^T  (= Sigma H^T)  [8,4]
    S   = H P12 + R                       [4,4]
    X   = S^-1  (SPD, 2x2-block Schur complement, fp32)
    KG  = P12 X                           [8,4]

Layout: "batch-innermost planes". The HOST pre-transposes every input to
[chunk, P, d0, d1, g] (P=128 SBUF partitions, g=64 consecutive batches
innermost) and casts to fp16. On device every per-batch product is a wide
elementwise DVE tensor_tensor in fp16 — the innermost g axis is stride-1 on
all operands, which enables the DVE 16-bit fast mode (~0.52 ns/free-elem
marginal + ~150 ns/op fixed, hence g=64 over g=32). Contraction sums for
A/U/P12/KG ride the TensorEngine (fp16 identity stationary, fp32 PSUM
accumulate, ~375 ns/512-col pass; ScalarE evacuates PSUM -> fp16). The HQ
products feed P12's PSUM directly through transposed access patterns. S
also sums on the PE (fp32 PSUM); DVE adds R while reading PSUM directly,
so S reaches the inverse in fp32. The 4x4 SPD inverse runs in fp32 on DVE,
batched over IBATCH chunks to amortize per-op overhead. GPSIMD is left idle
on purpose: concurrent GPSIMD traffic inflates DVE op durations ~40%
(SBUF port contention, measured).

Numerics (validated against a float64 reference in numpy): fp16 inputs +
fp16 products + fp32 PSUM sums + fp16 intermediates + fp32 S/inverse +
fp16 X/KG gives rel err ~3e-3 (tolerance 2e-2).
"""

import numpy as np

P = 128
G = 64
B = 262144
NCORES = 8
B_CORE = B // NCORES           # 32768
CHUNK = P * G                  # 8192
NCHUNK = B_CORE // CHUNK       # 4
IBATCH = 2                     # chunks per inverse batch

_NC_CACHE = {}


def _build_nc():
    import concourse.bacc as bacc
    import concourse.mybir as mybir
    import concourse.tile as tile
    from concourse.masks import make_identity

    fp32 = mybir.dt.float32
    fp16 = mybir.dt.float16
    MULT = mybir.AluOpType.mult
    ADD = mybir.AluOpType.add
    SUB = mybir.AluOpType.subtract
    COPY = mybir.ActivationFunctionType.Copy

    nc = bacc.Bacc("TRN2", target_bir_lowering=False, debug=False)

    F_d = nc.dram_tensor("F", [NCHUNK, P, 8, 8, G], fp16, kind="ExternalInput").ap()
    Sp_d = nc.dram_tensor(
        "Sigma_previous", [NCHUNK, P, 8, 8, G], fp16, kind="ExternalInput"
    ).ap()
    Q_d = nc.dram_tensor("Q", [NCHUNK, P, 8, 8, G], fp16, kind="ExternalInput").ap()
    H_d = nc.dram_tensor("H", [NCHUNK, P, 4, 8, G], fp16, kind="ExternalInput").ap()
    R_d = nc.dram_tensor("R", [NCHUNK, P, 4, 4, G], fp16, kind="ExternalInput").ap()
    KG_d = nc.dram_tensor("KG", [NCHUNK, P, 8, 4, G], fp16, kind="ExternalOutput").ap()

    NB = IBATCH
    W = 32 * G          # free elems per 32-entry matrix family (2048)
    NBANK = W // 512    # PSUM banks per family (4)

    with tile.TileContext(nc) as tc:
        with (
            tc.tile_pool(name="consts", bufs=1) as consts,
            tc.tile_pool(name="inF", bufs=4) as poolF,
            tc.tile_pool(name="inH", bufs=5) as poolH,
            tc.tile_pool(name="inSp", bufs=3) as poolSp,
            tc.tile_pool(name="inQ", bufs=2) as poolQ,
            tc.tile_pool(name="inR", bufs=2) as poolR,
            tc.tile_pool(name="pp", bufs=8) as ppool,
            tc.tile_pool(name="tprod", bufs=1) as tprod,
            tc.tile_pool(name="interm", bufs=2) as interm,
            tc.tile_pool(name="p12p", bufs=4) as p12p,
            tc.tile_pool(name="sx", bufs=2) as sxp,
            tc.tile_pool(name="inv", bufs=1) as invp,
            tc.tile_pool(name="out", bufs=2) as outp,
            tc.tile_pool(name="psum", bufs=2, space="PSUM") as psump,
        ):
            identf = consts.tile([P, P], fp32, tag="identf")
            make_identity(nc, identf[:])
            identh_t = consts.tile([P, P], fp16, tag="identh")
            nc.vector.tensor_copy(identh_t[:], identf[:])
            identh = identh_t[:]

            V = nc.vector
            ACT = nc.scalar

            def bc(ap, axis, shape):
                return ap.unsqueeze(axis).broadcast_to(shape)

            st = [dict() for _ in range(NCHUNK)]
            inv_st = [dict() for _ in range(NCHUNK // NB)]

            sh48 = [P, 4, 8, G]
            sh84 = [P, 8, 4, G]
            sh44 = [P, 4, 4, G]

            def emit_load(c):
                s = st[c]
                s["F"] = poolF.tile([P, 8, 8, G], fp16, tag="F", name="Ft")
                s["H"] = poolH.tile([P, 4, 8, G], fp16, tag="H", name="Ht")
                nc.sync.dma_start(out=s["H"][:], in_=H_d[c])
                if c == 0:
                    # split: A(0)'s first products only need F rows 0-3
                    nc.sync.dma_start(out=s["F"][:, 0:4], in_=F_d[c][:, 0:4])
                    nc.sync.dma_start(out=s["F"][:, 4:8], in_=F_d[c][:, 4:8])
                else:
                    nc.sync.dma_start(out=s["F"][:], in_=F_d[c])

            def emit_load_Sp(c):
                s = st[c]
                s["Sp"] = poolSp.tile([P, 8, 8, G], fp16, tag="Sp", name="Spt")
                nc.sync.dma_start(out=s["Sp"][:], in_=Sp_d[c])

            def emit_load_QR(c):
                s = st[c]
                s["Q"] = poolQ.tile([P, 8, 8, G], fp16, tag="Q", name="Qt")
                s["R"] = poolR.tile([P, 4, 4, G], fp16, tag="R", name="Rt")
                nc.sync.dma_start(out=s["Q"][:], in_=Q_d[c])
                nc.sync.dma_start(out=s["R"][:], in_=R_d[c])

            def pslot(shape):
                t = ppool.tile([P, W], fp16, tag="pp", name="pp")
                n = shape[1] * shape[2] * G
                return t[:][:, :n].rearrange(
                    "p (a b g) -> p a b g", a=shape[1], b=shape[2], g=G
                )

            def banks(ap4):
                # [P, d0, d1, G] -> NBANK 512-elem bank APs, split along d0
                d0 = ap4.shape[1]
                step = d0 // NBANK
                return [ap4[:, i * step : (i + 1) * step] for i in range(NBANK)]

            def pe_contract(slots, out_tag, pool):
                """slots: list of per-term [P, W]-wide fp16 moving APs.
                One W-wide matmul per term (spans W//512 PSUM banks), one
                W-wide ACT evacuation. Returns fp16 SBUF tile [P, W] flat."""
                nterm = len(slots)
                ps = psump.tile([P, W], fp32, tag="ps", name=f"ps_{out_tag}")
                for t, mv in enumerate(slots):
                    for b in range(NBANK):
                        nc.tensor.matmul(
                            ps[:, b * 512 : (b + 1) * 512],
                            identh,
                            mv[b],
                            start=(t == 0),
                            stop=(t == nterm - 1),
                        )
                out = pool.tile([P, W], fp16, tag=out_tag, name=out_tag)
                ACT.activation(out[:], ps[:, :], COPY)
                return out

            def emit_A(c):
                s = st[c]
                Ft, Ht = s["F"], s["H"]
                slots = []
                for j in range(8):
                    pv = pslot(sh48)
                    V.tensor_tensor(
                        pv,
                        bc(Ht[:, :, j, :], 2, sh48),
                        bc(Ft[:, j, :, :], 1, sh48),
                        op=MULT,
                    )
                    slots.append(banks(pv))
                s["A"] = pe_contract(slots, "A", interm)  # A[m,k]

            def emit_U(c):
                s = st[c]
                Spt = s["Sp"]
                Av = s["A"][:].rearrange("p (m k g) -> p m k g", m=4, k=8)
                slots = []
                for k in range(8):
                    pv = pslot(sh48)
                    V.tensor_tensor(
                        pv,
                        bc(Av[:, :, k, :], 2, sh48),
                        bc(Spt[:, k, :, :], 1, sh48),
                        op=MULT,
                    )
                    slots.append(banks(pv))
                s["U"] = pe_contract(slots, "U", interm)  # U[m,j]

            def emit_P12(c):
                s = st[c]
                Ft, Ht, Qt = s["F"], s["H"], s["Q"]
                Uv = s["U"][:].rearrange("p (m j g) -> p m j g", m=4, j=8)
                slots = []
                for j in range(8):
                    pv = pslot(sh84)
                    V.tensor_tensor(
                        pv,
                        bc(Ft[:, :, j, :], 2, sh84),
                        bc(Uv[:, :, j, :], 1, sh84),
                        op=MULT,
                    )
                    slots.append(banks(pv))
                # HQ products [m,k], fed transposed ([k,m]) into the same PSUM
                for j in range(8):
                    pv = pslot(sh48)
                    V.tensor_tensor(
                        pv,
                        bc(Ht[:, :, j, :], 2, sh48),
                        bc(Qt[:, j, :, :], 1, sh48),
                        op=MULT,
                    )
                    slots.append(banks(pv.transpose([0, 2, 1, 3])))
                s["P12"] = pe_contract(slots, "P12", p12p)

            def emit_S(c):
                s = st[c]
                Ht, Rt = s["H"], s["R"]
                P12v = s["P12"][:].rearrange("p (i m g) -> p i m g", i=8, m=4)
                WS = 16 * G
                NB2 = WS // 512
                ps = psump.tile([P, W], fp32, tag="ps", name="ps_S")
                for i in range(8):
                    pv = pslot(sh44)
                    V.tensor_tensor(
                        pv,
                        bc(Ht[:, :, i, :], 2, sh44),
                        bc(P12v[:, i, :, :], 1, sh44),
                        op=MULT,
                    )
                    for b in range(NB2):
                        nc.tensor.matmul(
                            ps[:, b * 512 : (b + 1) * 512],
                            identh,
                            pv[:, b * 2 : (b + 1) * 2],
                            start=(i == 0),
                            stop=False,
                        )
                # 9th term: R rides the PE too; ACT evacuates S4 (fp32) direct
                for b in range(NB2):
                    nc.tensor.matmul(
                        ps[:, b * 512 : (b + 1) * 512],
                        identh,
                        Rt[:, b * 2 : (b + 1) * 2],
                        start=False,
                        stop=True,
                    )
                k, ci = c // NB, c % NB
                if ci == 0:
                    inv_st[k]["S4"] = sxp.tile(
                        [P, 4, 4, NB, G], fp32, tag="S4", name="S4"
                    )
                S4 = inv_st[k]["S4"]
                ACT.activation(
                    S4[:, :, :, ci, :],
                    ps[:, :WS].rearrange("p (m n g) -> p m n g", m=4, n=4),
                    COPY,
                )

            def emit_INV(k):
                """X4 = S4^-1 via Schur complement of leading 2x2 block.
                S treated as symmetric (s10 := s01). All internals fp32,
                X4 written fp16. Scratch tags reused across steps (bufs=1)."""
                s = inv_st[k]
                S4 = s["S4"]
                X4 = sxp.tile([P, 4, 4, NB, G], fp16, tag="X4", name="X4")
                s["X4"] = X4
                sh1 = [P, NB, G]
                sh2 = [P, 2, 2, NB, G]

                def t1(tag):
                    return invp.tile(sh1, fp32, tag=tag, name=tag)

                def t2(tag):
                    return invp.tile(sh2, fp32, tag=tag, name=tag)

                sa, sb, sc_ = S4[:, 0, 0], S4[:, 0, 1], S4[:, 1, 1]
                ta, tb, td, tr, tnr = (
                    t1("ta"), t1("tb"), t1("td"), t1("tr"), t1("tnr"),
                )
                V.tensor_tensor(ta[:], sa, sc_, op=MULT)
                V.tensor_tensor(tb[:], sb, sb, op=MULT)
                V.tensor_tensor(td[:], ta[:], tb[:], op=SUB)
                V.reciprocal_approx_fast(tr[:], td[:])
                V.tensor_scalar_mul(tnr[:], tr[:], -1.0)
                Pi = t2("Pi")
                V.tensor_tensor(Pi[:, 0, 0], sc_, tr[:], op=MULT)
                V.tensor_tensor(Pi[:, 1, 1], sa, tr[:], op=MULT)
                V.tensor_tensor(Pi[:, 0, 1], sb, tnr[:], op=MULT)
                V.tensor_copy(Pi[:, 1, 0], Pi[:, 0, 1])
                Bq = S4[:, 2:4, 0:2]
                Wt, x2a = t2("W"), t2("x2a")
                V.tensor_tensor(
                    x2a[:], bc(Bq[:, :, 0], 2, sh2), bc(Pi[:, 0, :], 1, sh2), op=MULT
                )
                V.tensor_tensor(
                    Wt[:], bc(Bq[:, :, 1], 2, sh2), bc(Pi[:, 1, :], 1, sh2), op=MULT
                )
                V.tensor_tensor(Wt[:], Wt[:], x2a[:], op=ADD)
                x2b, Sc = t2("x2b"), t2("Sc")
                V.tensor_tensor(
                    x2a[:], bc(Wt[:, :, 0], 2, sh2), bc(Bq[:, :, 0], 1, sh2), op=MULT
                )
                V.tensor_tensor(
                    x2b[:], bc(Wt[:, :, 1], 2, sh2), bc(Bq[:, :, 1], 1, sh2), op=MULT
                )
                V.tensor_tensor(x2a[:], x2a[:], x2b[:], op=ADD)
                V.tensor_tensor(Sc[:], S4[:, 2:4, 2:4], x2a[:], op=SUB)
                V.tensor_tensor(ta[:], Sc[:, 0, 0], Sc[:, 1, 1], op=MULT)
                V.tensor_tensor(tb[:], Sc[:, 0, 1], Sc[:, 1, 0], op=MULT)
                V.tensor_tensor(td[:], ta[:], tb[:], op=SUB)
                V.reciprocal_approx_fast(tr[:], td[:])
                V.tensor_scalar_mul(tnr[:], tr[:], -1.0)
                Si = t2("Si")
                V.tensor_tensor(Si[:, 0, 0], Sc[:, 1, 1], tr[:], op=MULT)
                V.tensor_tensor(Si[:, 1, 1], Sc[:, 0, 0], tr[:], op=MULT)
                V.tensor_tensor(Si[:, 0, 1], Sc[:, 0, 1], tnr[:], op=MULT)
                V.tensor_copy(Si[:, 1, 0], Si[:, 0, 1])
                ACT.activation(X4[:, 2:4, 2:4], Si[:], COPY)
                x2c = t2("Sc")  # Sc is dead past this point; reuse its buffer
                V.tensor_tensor(
                    x2a[:], bc(Si[:, :, 0], 2, sh2), bc(Wt[:, 0, :], 1, sh2), op=MULT
                )
                V.tensor_tensor(
                    x2b[:], bc(Si[:, :, 1], 2, sh2), bc(Wt[:, 1, :], 1, sh2), op=MULT
                )
                V.tensor_tensor(x2a[:], x2a[:], x2b[:], op=ADD)
                ACT.activation(X4[:, 2:4, 0:2], x2a[:], COPY, scale=-1.0)
                ACT.activation(
                    X4[:, 0:2, 2:4],
                    X4[:, 2:4, 0:2].transpose([0, 2, 1, 3, 4]),
                    COPY,
                )
                V.tensor_tensor(
                    x2b[:], bc(Wt[:, 0, :], 2, sh2), bc(x2a[:, 0, :], 1, sh2), op=MULT
                )
                V.tensor_tensor(
                    x2c[:], bc(Wt[:, 1, :], 2, sh2), bc(x2a[:, 1, :], 1, sh2), op=MULT
                )
                V.tensor_tensor(x2b[:], x2b[:], x2c[:], op=ADD)
                V.tensor_tensor(X4[:, 0:2, 0:2], Pi[:], x2b[:], op=ADD)

            def emit_KG(c):
                s = st[c]
                k, ci = c // NB, c % NB
                X4 = inv_st[k]["X4"]
                P12v = s["P12"][:].rearrange("p (i m g) -> p i m g", i=8, m=4)
                slots = []
                for m in range(4):
                    pv = pslot(sh84)
                    V.tensor_tensor(
                        pv,
                        bc(P12v[:, :, m, :], 2, sh84),
                        bc(X4[:, m, :, ci, :], 1, sh84),
                        op=MULT,
                    )
                    slots.append(banks(pv))
                ps = psump.tile([P, W], fp32, tag="ps", name="ps_KG")
                for t, mv in enumerate(slots):
                    for b in range(NBANK):
                        nc.tensor.matmul(
                            ps[:, b * 512 : (b + 1) * 512], identh, mv[b],
                            start=(t == 0), stop=(t == 3),
                        )
                KGh = outp.tile([P, 8, 4, G], fp16, tag="KGh", name="KGh")
                ACT.activation(KGh[:].rearrange("p i n g -> p (i n g)"), ps[:, :], COPY)
                nc.sync.dma_start(out=KG_d[c], in_=KGh[:])

            for t in range(NCHUNK + 7):
                if t < NCHUNK:
                    emit_load(t)
                if t < NCHUNK:
                    emit_load_Sp(t)
                if 0 <= t - 2 < NCHUNK:
                    emit_load_QR(t - 2)
                if 0 <= t - 1 < NCHUNK:
                    emit_A(t - 1)
                if 0 <= t - 2 < NCHUNK:
                    emit_U(t - 2)
                if 0 <= t - 3 < NCHUNK:
                    emit_P12(t - 3)
                if 0 <= t - 4 < NCHUNK:
                    emit_S(t - 4)
                if 0 <= t - 4 < NCHUNK and (t - 4) % NB == NB - 1:
                    emit_INV((t - 4) // NB)
                if 0 <= t - 6 < NCHUNK:
                    emit_KG(t - 6)

    nc.compile()
    return nc


def _get_nc():
    if "nc" not in _NC_CACHE:
        _NC_CACHE["nc"] = _build_nc()
    return _NC_CACHE["nc"]


def prepare_in_map(F, H, Sigma_previous, Q, R, core):
    """Host-side shard + layout transform + fp16 cast for one core."""
    sl = slice(core * B_CORE, (core + 1) * B_CORE)

    def bi(x, d0, d1):
        # [B_CORE, d0, d1] -> [NCHUNK, P, d0, d1, G]
        v = x[sl].reshape(NCHUNK, P, G, d0, d1).transpose(0, 1, 3, 4, 2)
        return np.ascontiguousarray(v, dtype=np.float16)

    return {
        "F": bi(F, 8, 8),
        "Sigma_previous": bi(Sigma_previous, 8, 8),
        "Q": bi(Q, 8, 8),
        "H": bi(H, 4, 8),
        "R": bi(R, 4, 4),
    }


def postprocess(results):
    """[per-core dicts with KG [NCHUNK, P, 8, 4, G] fp16] -> [B, 8, 4] fp32."""
    outs = []
    for r in results:
        kg = r["KG"].astype(np.float32)          # [NCHUNK, P, 8, 4, G]
        kg = kg.transpose(0, 1, 4, 2, 3).reshape(B_CORE, 8, 4)
        outs.append(kg)
    return np.concatenate(outs, axis=0)


def kernel(F, H, Sigma_previous, Q, R):
    from concourse.bass_utils import run_bass_kernel_spmd

    nc = _get_nc()
    in_maps = [
        prepare_in_map(F, H, Sigma_previous, Q, R, ci) for ci in range(NCORES)
    ]
    res = run_bass_kernel_spmd(nc, in_maps, core_ids=list(range(NCORES)))
    return postprocess(res.results)


# revision 33
# speedup vs baseline: 1.0383x; 1.0006x over previous
"""Batched Kalman-gain kernel for Trainium2 (Bass/Tile), 8-core data parallel.

Per batch b (262144 of them):
    Sigma = F Sp F^T + Q            [8,8]
    S     = H Sigma H^T + R         [4,4]
    KG    = Sigma H^T S^-1          [8,4]

Factored (A = H F, U = A Sp):
    P12 = F U^T + (H Q)^T  (= Sigma H^T)  [8,4]
    S   = H P12 + R                       [4,4]
    X   = S^-1  (SPD, 2x2-block Schur complement, fp32)
    KG  = P12 X                           [8,4]

Layout: "batch-innermost planes". The HOST pre-transposes every input to
[chunk, P, d0, d1, g] (P=128 SBUF partitions, g=64 consecutive batches
innermost) and casts to fp16. On device every per-batch product is a wide
elementwise DVE tensor_tensor in fp16 — the innermost g axis is stride-1 on
all operands, which enables the DVE 16-bit fast mode (~0.52 ns/free-elem
marginal + ~150 ns/op fixed, hence g=64 over g=32). Contraction sums for
A/U/P12/KG ride the TensorEngine (fp16 identity stationary, fp32 PSUM
accumulate, ~375 ns/512-col pass; ScalarE evacuates PSUM -> fp16). The HQ
products feed P12's PSUM directly through transposed access patterns. S
also sums on the PE (fp32 PSUM); DVE adds R while reading PSUM directly,
so S reaches the inverse in fp32. The 4x4 SPD inverse runs in fp32 on DVE,
batched over IBATCH chunks to amortize per-op overhead. GPSIMD is left idle
on purpose: concurrent GPSIMD traffic inflates DVE op durations ~40%
(SBUF port contention, measured).

Numerics (validated against a float64 reference in numpy): fp16 inputs +
fp16 products + fp32 PSUM sums + fp16 intermediates + fp32 S/inverse +
fp16 X/KG gives rel err ~3e-3 (tolerance 2e-2).
"""

import numpy as np

P = 128
G = 64
B = 262144
NCORES = 8
B_CORE = B // NCORES           # 32768
CHUNK = P * G                  # 8192
NCHUNK = B_CORE // CHUNK       # 4
IBATCH = 2                     # chunks per inverse batch

_NC_CACHE = {}


def _build_nc():
    import concourse.bacc as bacc
    import concourse.mybir as mybir
    import concourse.tile as tile
    from concourse.masks import make_identity

    fp32 = mybir.dt.float32
    fp16 = mybir.dt.float16
    MULT = mybir.AluOpType.mult
    ADD = mybir.AluOpType.add
    SUB = mybir.AluOpType.subtract
    COPY = mybir.ActivationFunctionType.Copy

    nc = bacc.Bacc("TRN2", target_bir_lowering=False, debug=False)

    F_d = nc.dram_tensor("F", [NCHUNK, P, 8, 8, G], fp16, kind="ExternalInput").ap()
    Sp_d = nc.dram_tensor(
        "Sigma_previous", [NCHUNK, P, 8, 8, G], fp16, kind="ExternalInput"
    ).ap()
    Q_d = nc.dram_tensor("Q", [NCHUNK, P, 8, 8, G], fp16, kind="ExternalInput").ap()
    H_d = nc.dram_tensor("H", [NCHUNK, P, 4, 8, G], fp16, kind="ExternalInput").ap()
    R_d = nc.dram_tensor("R", [NCHUNK, P, 4, 4, G], fp16, kind="ExternalInput").ap()
    KG_d = nc.dram_tensor("KG", [NCHUNK, P, 8, 4, G], fp16, kind="ExternalOutput").ap()

    NB = IBATCH
    W = 32 * G          # free elems per 32-entry matrix family (2048)
    NBANK = W // 512    # PSUM banks per family (4)

    with tile.TileContext(nc) as tc:
        with (
            tc.tile_pool(name="consts", bufs=1) as consts,
            tc.tile_pool(name="inF", bufs=4) as poolF,
            tc.tile_pool(name="inH", bufs=5) as poolH,
            tc.tile_pool(name="inSp", bufs=3) as poolSp,
            tc.tile_pool(name="inQ", bufs=2) as poolQ,
            tc.tile_pool(name="inR", bufs=2) as poolR,
            tc.tile_pool(name="pp", bufs=8) as ppool,
            tc.tile_pool(name="tprod", bufs=1) as tprod,
            tc.tile_pool(name="interm", bufs=2) as interm,
            tc.tile_pool(name="p12p", bufs=4) as p12p,
            tc.tile_pool(name="sx", bufs=2) as sxp,
            tc.tile_pool(name="inv", bufs=1) as invp,
            tc.tile_pool(name="out", bufs=2) as outp,
            tc.tile_pool(name="psum", bufs=2, space="PSUM") as psump,
        ):
            identf = consts.tile([P, P], fp32, tag="identf")
            make_identity(nc, identf[:])
            identh_t = consts.tile([P, P], fp16, tag="identh")
            nc.vector.tensor_copy(identh_t[:], identf[:])
            identh = identh_t[:]

            V = nc.vector
            ACT = nc.scalar

            def bc(ap, axis, shape):
                return ap.unsqueeze(axis).broadcast_to(shape)

            st = [dict() for _ in range(NCHUNK)]
            inv_st = [dict() for _ in range(NCHUNK // NB)]

            sh48 = [P, 4, 8, G]
            sh84 = [P, 8, 4, G]
            sh44 = [P, 4, 4, G]

            def emit_load(c):
                s = st[c]
                s["F"] = poolF.tile([P, 8, 8, G], fp16, tag="F", name="Ft")
                s["H"] = poolH.tile([P, 4, 8, G], fp16, tag="H", name="Ht")
                nc.sync.dma_start(out=s["H"][:], in_=H_d[c])
                if c == 0:
                    # split: A(0)'s first products only need F rows 0-3
                    nc.sync.dma_start(out=s["F"][:, 0:4], in_=F_d[c][:, 0:4])
                    nc.sync.dma_start(out=s["F"][:, 4:8], in_=F_d[c][:, 4:8])
                else:
                    nc.sync.dma_start(out=s["F"][:], in_=F_d[c])

            def emit_load_Sp(c):
                s = st[c]
                s["Sp"] = poolSp.tile([P, 8, 8, G], fp16, tag="Sp", name="Spt")
                nc.sync.dma_start(out=s["Sp"][:], in_=Sp_d[c])

            def emit_load_QR(c):
                s = st[c]
                s["Q"] = poolQ.tile([P, 8, 8, G], fp16, tag="Q", name="Qt")
                s["R"] = poolR.tile([P, 4, 4, G], fp16, tag="R", name="Rt")
                nc.sync.dma_start(out=s["Q"][:], in_=Q_d[c])
                nc.sync.dma_start(out=s["R"][:], in_=R_d[c])

            def pslot(shape):
                t = ppool.tile([P, W], fp16, tag="pp", name="pp")
                n = shape[1] * shape[2] * G
                return t[:][:, :n].rearrange(
                    "p (a b g) -> p a b g", a=shape[1], b=shape[2], g=G
                )

            def banks(ap4):
                # [P, d0, d1, G] -> NBANK 512-elem bank APs, split along d0
                d0 = ap4.shape[1]
                step = d0 // NBANK
                return [ap4[:, i * step : (i + 1) * step] for i in range(NBANK)]

            def pe_contract(slots, out_tag, pool):
                """slots: list of per-term [P, W]-wide fp16 moving APs.
                One W-wide matmul per term (spans W//512 PSUM banks), one
                W-wide ACT evacuation. Returns fp16 SBUF tile [P, W] flat."""
                nterm = len(slots)
                ps = psump.tile([P, W], fp32, tag="ps", name=f"ps_{out_tag}")
                for t, mv in enumerate(slots):
                    for b in range(NBANK):
                        nc.tensor.matmul(
                            ps[:, b * 512 : (b + 1) * 512],
                            identh,
                            mv[b],
                            start=(t == 0),
                            stop=(t == nterm - 1),
                        )
                out = pool.tile([P, W], fp16, tag=out_tag, name=out_tag)
                ACT.activation(out[:], ps[:, :], COPY)
                return out

            def emit_A(c):
                s = st[c]
                Ft, Ht = s["F"], s["H"]
                slots = []
                for j in range(8):
                    pv = pslot(sh48)
                    V.tensor_tensor(
                        pv,
                        bc(Ht[:, :, j, :], 2, sh48),
                        bc(Ft[:, j, :, :], 1, sh48),
                        op=MULT,
                    )
                    slots.append(banks(pv))
                s["A"] = pe_contract(slots, "A", interm)  # A[m,k]

            def emit_U(c):
                s = st[c]
                Spt = s["Sp"]
                Av = s["A"][:].rearrange("p (m k g) -> p m k g", m=4, k=8)
                slots = []
                for k in range(8):
                    pv = pslot(sh48)
                    V.tensor_tensor(
                        pv,
                        bc(Av[:, :, k, :], 2, sh48),
                        bc(Spt[:, k, :, :], 1, sh48),
                        op=MULT,
                    )
                    slots.append(banks(pv))
                s["U"] = pe_contract(slots, "U", interm)  # U[m,j]

            def emit_P12(c):
                s = st[c]
                Ft, Ht, Qt = s["F"], s["H"], s["Q"]
                Uv = s["U"][:].rearrange("p (m j g) -> p m j g", m=4, j=8)
                slots = []
                for j in range(8):
                    pv = pslot(sh84)
                    V.tensor_tensor(
                        pv,
                        bc(Ft[:, :, j, :], 2, sh84),
                        bc(Uv[:, :, j, :], 1, sh84),
                        op=MULT,
                    )
                    slots.append(banks(pv))
                # HQ products [m,k], fed transposed ([k,m]) into the same PSUM
                for j in range(8):
                    pv = pslot(sh48)
                    V.tensor_tensor(
                        pv,
                        bc(Ht[:, :, j, :], 2, sh48),
                        bc(Qt[:, j, :, :], 1, sh48),
                        op=MULT,
                    )
                    slots.append(banks(pv.transpose([0, 2, 1, 3])))
                s["P12"] = pe_contract(slots, "P12", p12p)

            def emit_S(c):
                s = st[c]
                Ht, Rt = s["H"], s["R"]
                P12v = s["P12"][:].rearrange("p (i m g) -> p i m g", i=8, m=4)
                WS = 16 * G
                NB2 = WS // 512
                ps = psump.tile([P, W], fp32, tag="ps", name="ps_S")
                for i in range(8):
                    pv = pslot(sh44)
                    V.tensor_tensor(
                        pv,
                        bc(Ht[:, :, i, :], 2, sh44),
                        bc(P12v[:, i, :, :], 1, sh44),
                        op=MULT,
                    )
                    for b in range(NB2):
                        nc.tensor.matmul(
                            ps[:, b * 512 : (b + 1) * 512],
                            identh,
                            pv[:, b * 2 : (b + 1) * 2],
                            start=(i == 0),
                            stop=False,
                        )
                # 9th term: R rides the PE too; ACT evacuates S4 (fp32) direct
                for b in range(NB2):
                    nc.tensor.matmul(
                        ps[:, b * 512 : (b + 1) * 512],
                        identh,
                        Rt[:, b * 2 : (b + 1) * 2],
                        start=False,
                        stop=True,
                    )
                k, ci = c // NB, c % NB
                if ci == 0:
                    inv_st[k]["S4"] = sxp.tile(
                        [P, 4, 4, NB, G], fp32, tag="S4", name="S4"
                    )
                S4 = inv_st[k]["S4"]
                ACT.activation(
                    S4[:, :, :, ci, :],
                    ps[:, :WS].rearrange("p (m n g) -> p m n g", m=4, n=4),
                    COPY,
                )

            def emit_INV(k):
                """X4 = S4^-1 via Schur complement of leading 2x2 block.
                S treated as symmetric (s10 := s01). All internals fp32,
                X4 written fp16. Scratch tags reused across steps (bufs=1)."""
                s = inv_st[k]
                S4 = s["S4"]
                X4 = sxp.tile([P, 4, 4, NB, G], fp16, tag="X4", name="X4")
                s["X4"] = X4
                sh1 = [P, NB, G]
                sh2 = [P, 2, 2, NB, G]

                def t1(tag):
                    return invp.tile(sh1, fp32, tag=tag, name=tag)

                def t2(tag):
                    return invp.tile(sh2, fp32, tag=tag, name=tag)

                sa, sb, sc_ = S4[:, 0, 0], S4[:, 0, 1], S4[:, 1, 1]
                ta, tb, td, tr, tnr = (
                    t1("ta"), t1("tb"), t1("td"), t1("tr"), t1("tnr"),
                )
                V.tensor_tensor(ta[:], sa, sc_, op=MULT)
                V.tensor_tensor(tb[:], sb, sb, op=MULT)
                V.tensor_tensor(td[:], ta[:], tb[:], op=SUB)
                V.reciprocal_approx_fast(tr[:], td[:])
                V.tensor_scalar_mul(tnr[:], tr[:], -1.0)
                Pi = t2("Pi")
                V.tensor_tensor(Pi[:, 0, 0], sc_, tr[:], op=MULT)
                V.tensor_tensor(Pi[:, 1, 1], sa, tr[:], op=MULT)
                V.tensor_tensor(Pi[:, 0, 1], sb, tnr[:], op=MULT)
                V.tensor_copy(Pi[:, 1, 0], Pi[:, 0, 1])
                Bq = S4[:, 2:4, 0:2]
                Wt, x2a = t2("W"), t2("x2a")
                V.tensor_tensor(
                    x2a[:], bc(Bq[:, :, 0], 2, sh2), bc(Pi[:, 0, :], 1, sh2), op=MULT
                )
                V.tensor_tensor(
                    Wt[:], bc(Bq[:, :, 1], 2, sh2), bc(Pi[:, 1, :], 1, sh2), op=MULT
                )
                V.tensor_tensor(Wt[:], Wt[:], x2a[:], op=ADD)
                x2b, Sc = t2("x2b"), t2("Sc")
                V.tensor_tensor(
                    x2a[:], bc(Wt[:, :, 0], 2, sh2), bc(Bq[:, :, 0], 1, sh2), op=MULT
                )
                V.tensor_tensor(
                    x2b[:], bc(Wt[:, :, 1], 2, sh2), bc(Bq[:, :, 1], 1, sh2), op=MULT
                )
                V.tensor_tensor(x2a[:], x2a[:], x2b[:], op=ADD)
                V.tensor_tensor(Sc[:], S4[:, 2:4, 2:4], x2a[:], op=SUB)
                V.tensor_tensor(ta[:], Sc[:, 0, 0], Sc[:, 1, 1], op=MULT)
                V.tensor_tensor(tb[:], Sc[:, 0, 1], Sc[:, 1, 0], op=MULT)
                V.tensor_tensor(td[:], ta[:], tb[:], op=SUB)
                V.reciprocal_approx_fast(tr[:], td[:])
                V.tensor_scalar_mul(tnr[:], tr[:], -1.0)
                Si = t2("Si")
                V.tensor_tensor(Si[:, 0, 0], Sc[:, 1, 1], tr[:], op=MULT)
                V.tensor_tensor(Si[:, 1, 1], Sc[:, 0, 0], tr[:], op=MULT)
                V.tensor_tensor(Si[:, 0, 1], Sc[:, 0, 1], tnr[:], op=MULT)
                V.tensor_copy(Si[:, 1, 0], Si[:, 0, 1])
                ACT.activation(X4[:, 2:4, 2:4], Si[:], COPY)
                x2c = t2("Sc")  # Sc is dead past this point; reuse its buffer
                V.tensor_tensor(
                    x2a[:], bc(Si[:, :, 0], 2, sh2), bc(Wt[:, 0, :], 1, sh2), op=MULT
                )
                V.tensor_tensor(
                    x2b[:], bc(Si[:, :, 1], 2, sh2), bc(Wt[:, 1, :], 1, sh2), op=MULT
                )
                V.tensor_tensor(x2a[:], x2a[:], x2b[:], op=ADD)
                ACT.activation(X4[:, 2:4, 0:2], x2a[:], COPY, scale=-1.0)
                ACT.activation(
                    X4[:, 0:2, 2:4],
                    X4[:, 2:4, 0:2].transpose([0, 2, 1, 3, 4]),
                    COPY,
                )
                V.tensor_tensor(
                    x2b[:], bc(Wt[:, 0, :], 2, sh2), bc(x2a[:, 0, :], 1, sh2), op=MULT
                )
                V.tensor_tensor(
                    x2c[:], bc(Wt[:, 1, :], 2, sh2), bc(x2a[:, 1, :], 1, sh2), op=MULT
                )
                V.tensor_tensor(x2b[:], x2b[:], x2c[:], op=ADD)
                V.tensor_tensor(X4[:, 0:2, 0:2], Pi[:], x2b[:], op=ADD)

            def emit_KG(c):
                s = st[c]
                k, ci = c // NB, c % NB
                X4 = inv_st[k]["X4"]
                P12v = s["P12"][:].rearrange("p (i m g) -> p i m g", i=8, m=4)
                slots = []
                for m in range(4):
                    pv = pslot(sh84)
                    V.tensor_tensor(
                        pv,
                        bc(P12v[:, :, m, :], 2, sh84),
                        bc(X4[:, m, :, ci, :], 1, sh84),
                        op=MULT,
                    )
                    slots.append(banks(pv))
                ps = psump.tile([P, W], fp32, tag="ps", name="ps_KG")
                for t, mv in enumerate(slots):
                    for b in range(NBANK):
                        nc.tensor.matmul(
                            ps[:, b * 512 : (b + 1) * 512], identh, mv[b],
                            start=(t == 0), stop=(t == 3),
                        )
                KGh = outp.tile([P, 8, 4, G], fp16, tag="KGh", name="KGh")
                ACT.activation(KGh[:].rearrange("p i n g -> p (i n g)"), ps[:, :], COPY)
                nc.sync.dma_start(out=KG_d[c], in_=KGh[:])

            for t in range(NCHUNK + 7):
                if t < NCHUNK:
                    emit_load(t)
                if t < NCHUNK:
                    emit_load_Sp(t)
                if 0 <= t - 2 < NCHUNK:
                    emit_load_QR(t - 2)
                if 0 <= t - 1 < NCHUNK:
                    emit_A(t - 1)
                if 0 <= t - 2 < NCHUNK:
                    emit_U(t - 2)
                if 0 <= t - 3 < NCHUNK:
                    emit_P12(t - 3)
                if 0 <= t - 4 < NCHUNK:
                    emit_S(t - 4)
                if 0 <= t - 4 < NCHUNK and (t - 4) % NB == NB - 1:
                    emit_INV((t - 4) // NB)
                if 0 <= t - 6 < NCHUNK:
                    emit_KG(t - 6)

    nc.compile()
    return nc


def _get_nc():
    if "nc" not in _NC_CACHE:
        _NC_CACHE["nc"] = _build_nc()
    return _NC_CACHE["nc"]


def prepare_in_map(F, H, Sigma_previous, Q, R, core):
    """Host-side shard + layout transform + fp16 cast for one core."""
    sl = slice(core * B_CORE, (core + 1) * B_CORE)

    def bi(x, d0, d1):
        # [B_CORE, d0, d1] -> [NCHUNK, P, d0, d1, G]
        v = x[sl].reshape(NCHUNK, P, G, d0, d1).transpose(0, 1, 3, 4, 2)
        return np.ascontiguousarray(v, dtype=np.float16)

    return {
        "F": bi(F, 8, 8),
        "Sigma_previous": bi(Sigma_previous, 8, 8),
        "Q": bi(Q, 8, 8),
        "H": bi(H, 4, 8),
        "R": bi(R, 4, 4),
    }


def postprocess(results):
    """[per-core dicts with KG [NCHUNK, P, 8, 4, G] fp16] -> [B, 8, 4] fp32."""
    outs = []
    for r in results:
        kg = r["KG"].astype(np.float32)          # [NCHUNK, P, 8, 4, G]
        kg = kg.transpose(0, 1, 4, 2, 3).reshape(B_CORE, 8, 4)
        outs.append(kg)
    return np.concatenate(outs, axis=0)


def kernel(F, H, Sigma_previous, Q, R):
    from concourse.bass_utils import run_bass_kernel_spmd

    nc = _get_nc()
    in_maps = [
        prepare_in_map(F, H, Sigma_previous, Q, R, ci) for ci in range(NCORES)
    ]
    res = run_bass_kernel_spmd(nc, in_maps, core_ids=list(range(NCORES)))
    return postprocess(res.results)
